# revision 1
# baseline (speedup 1.0000x reference)
"""Cross-multi-head-attention (causal) Trainium2 Bass kernel.

Problem: B=4, T=2048, C=2048, 16 heads x head_dim 128.
  kv = enc_x @ W_kv + b_kv ; q = dec_x @ W_q + b_q
  out = softmax_causal(q k^T / sqrt(hd)) v  -> concat heads -> @ W_o + b_o

Sharding over 8 cores: core c -> (batch b = c//2, head-group hg = c%2 of 8
heads). Each core computes its 8 heads' K/V/Q projections, causal attention,
and a partial output projection (rows of W_o for its heads). Host sums the
two partials per batch and adds b_o.

All matmuls run as float32r (TF32-like: full PE rate, ~1.6e-4 rel err).
Layout strategy: host passes X^T so K^T/Q^T (head-dim on partitions) and
V (natural) are produced directly by matmuls with no on-chip transposes.
Attention computes S^T = K_h x Q_h^T tiles [k 128 x q 512]; softmax runs
without max-subtraction (scores are O(6) for this data); denominators come
from an accumulating ones-matmul partition reduction on the PE; the PV
matmul consumes unnormalized exp tiles and O^T is normalized afterwards
(reciprocal on DVE, broadcast via a rank-1 PE outer product).
"""
import sys

sys.path.insert(0, "/opt/trn_rl_repo")

import numpy as np

DIM = 2048
N_HEAD = 16
HEAD = DIM // N_HEAD  # 128
B = 4
T = 2048
HPC = 8               # heads per core
KC = HPC * HEAD       # 1024 projected cols per core
SCALE = 1.0 / np.sqrt(float(HEAD))
N_CORES = 8


def _build(t=T, add_bias_kq=False):
    from contextlib import ExitStack

    import concourse.mybir as mybir
    from concourse import bacc
    from concourse.tile import TileContext

    F32 = mybir.dt.float32
    F32R = mybir.dt.float32r
    AF = mybir.ActivationFunctionType

    n_tb = t // 512      # 512-col T blocks
    n_tc = t // 128      # 128-row T chunks
    n_g = t // 512       # q groups in attention

    nc = bacc.Bacc("TRN2", target_bir_lowering=False, debug=False, num_devices=1)
    xeT = nc.dram_tensor("xeT", [DIM, t], F32R, kind="ExternalInput").ap()
    xdT = nc.dram_tensor("xdT", [DIM, t], F32R, kind="ExternalInput").ap()
    wk = nc.dram_tensor("wk", [DIM, KC], F32R, kind="ExternalInput").ap()
    wv = nc.dram_tensor("wv", [DIM, KC], F32R, kind="ExternalInput").ap()
    wq = nc.dram_tensor("wq", [DIM, KC], F32R, kind="ExternalInput").ap()
    wo = nc.dram_tensor("wo", [KC, DIM], F32R, kind="ExternalInput").ap()
    masks = nc.dram_tensor("masks", [4, 128, 512], F32, kind="ExternalInput").ap()
    ones = nc.dram_tensor("ones", [128, 1], F32R, kind="ExternalInput").ap()
    ones_r = nc.dram_tensor("ones_r", [1, 128], F32R, kind="ExternalInput").ap()
    if add_bias_kq:
        bk = nc.dram_tensor("bk", [KC, 1], F32, kind="ExternalInput").ap()
        bq = nc.dram_tensor("bq", [KC, 1], F32, kind="ExternalInput").ap()
        bvb = nc.dram_tensor("bvb", [128, KC], F32, kind="ExternalInput").ap()
    out = nc.dram_tensor("out", [t, DIM], F32, kind="ExternalOutput").ap()

    kt_s = nc.dram_tensor("kt_s", [KC, t], F32R, kind="Internal").ap()
    qt_s = nc.dram_tensor("qt_s", [KC, t], F32R, kind="Internal").ap()
    v_s = nc.dram_tensor("v_s", [t, KC], F32R, kind="Internal").ap()
    ot_s = nc.dram_tensor("ot_s", [KC, t], F32R, kind="Internal").ap()

    with TileContext(nc) as tc, ExitStack() as top:
        glob = top.enter_context(tc.tile_pool(name="glob", bufs=1))
        mask_sb = []
        for o in range(4):
            m = glob.tile([128, 512], F32, tag=f"mask{o}", name=f"mask{o}")
            nc.sync.dma_start(out=m, in_=masks[o])
            mask_sb.append(m)
        ones_sb = glob.tile([128, 1], F32R, tag="ones", name="ones_sb")
        nc.sync.dma_start(out=ones_sb, in_=ones)
        ones_r_sb = glob.tile([1, 128], F32R, tag="onesr", name="ones_r_sb")
        nc.sync.dma_start(out=ones_r_sb, in_=ones_r)
        bk_b = bq_b = bvb_sb = None
        if add_bias_kq:
            bk_sb = glob.tile([128, HPC], F32, tag="bk", name="bk_sb")
            bq_sb = glob.tile([128, HPC], F32, tag="bq", name="bq_sb")
            bvb_sb = glob.tile([128, KC], F32, tag="bvb", name="bvb_sb")
            for h in range(HPC):
                nc.sync.dma_start(out=bk_sb[:, h:h + 1],
                                  in_=bk[h * 128:(h + 1) * 128, :])
                nc.sync.dma_start(out=bq_sb[:, h:h + 1],
                                  in_=bq[h * 128:(h + 1) * 128, :])
            nc.sync.dma_start(out=bvb_sb, in_=bvb)
            bk_b = [bk_sb[:, h:h + 1] for h in range(HPC)]
            bq_b = [bq_sb[:, h:h + 1] for h in range(HPC)]

        # attention kt/v pools open early (left side) so head-0 K^T and V
        # tiles prefetch during the projection phases
        att = ExitStack()
        aktp = att.enter_context(tc.tile_pool(name="akt", bufs=8))
        aqtp = att.enter_context(tc.tile_pool(name="aqt", bufs=8))
        avp = att.enter_context(tc.tile_pool(name="av", bufs=32))

        # ---- projection pools (shared across K, V, Q phases) ----
        proj = ExitStack()
        xp = proj.enter_context(tc.tile_pool(name="px", bufs=28, side="right"))
        wp = proj.enter_context(tc.tile_pool(name="pw", bufs=16, side="right"))
        op = proj.enter_context(tc.tile_pool(name="pop", bufs=6, side="right"))
        pp = proj.enter_context(tc.tile_pool(name="pps", bufs=8, space="PSUM"))

        def kq_proj(xT, w, out_scratch, bias_sb, pfx):
            wts = []
            xt0 = []
            for c in range(16):
                wt = wp.tile([128, KC], F32R, tag="w", name=f"{pfx}w_{c}")
                nc.sync.dma_start(out=wt, in_=w[c * 128:(c + 1) * 128, :])
                wts.append(wt)
                x1 = xp.tile([128, 512], F32R, tag="x", name=f"{pfx}x0_{c}")
                nc.sync.dma_start(out=x1, in_=xT[c * 128:(c + 1) * 128, 0:512])
                xt0.append(x1)
            for tb in range(n_tb):
                if tb == 0:
                    xt = xt0
                else:
                    xt = []
                    for c in range(16):
                        x1 = xp.tile([128, 512], F32R, tag="x",
                                     name=f"{pfx}x{tb}_{c}")
                        nc.sync.dma_start(
                            out=x1,
                            in_=xT[c * 128:(c + 1) * 128, tb * 512:(tb + 1) * 512])
                        xt.append(x1)
                ps = [pp.tile([128, 512], F32, tag="p", name=f"{pfx}p{tb}_{h}")
                      for h in range(HPC)]
                for c in range(16):
                    for h in range(HPC):
                        nc.tensor.matmul(
                            ps[h], wts[c][:, h * 128:(h + 1) * 128], xt[c],
                            start=(c == 0), stop=(c == 15))
                for h in range(HPC):
                    ot = op.tile([128, 512], F32R, tag="o", name=f"{pfx}o{tb}_{h}")
                    if bias_sb is not None:
                        nc.scalar.activation(ot, ps[h], AF.Identity,
                                             bias=bias_sb[h])
                    else:
                        nc.vector.tensor_copy(ot, ps[h])
                    nc.sync.dma_start(
                        out=out_scratch[h * 128:(h + 1) * 128,
                                        tb * 512:(tb + 1) * 512],
                        in_=ot)

        def v_proj():
            wts = []
            for c in range(16):
                wt = wp.tile([128, KC], F32R, tag="w", name=f"vw_{c}")
                nc.sync.dma_start(out=wt, in_=wv[c * 128:(c + 1) * 128, :])
                wts.append(wt)
            for tb in range(n_tb):
                xt = []
                for c in range(16):
                    x1 = xp.tile([128, 512], F32R, tag="x", name=f"vx{tb}_{c}")
                    nc.sync.dma_start(
                        out=x1,
                        in_=xeT[c * 128:(c + 1) * 128, tb * 512:(tb + 1) * 512])
                    xt.append(x1)
                ps = [pp.tile([128, 512], F32, tag="p", name=f"vp{tb}_{j}")
                      for j in range(8)]
                for c in range(16):
                    for ts in range(4):
                        for vg in range(2):
                            nc.tensor.matmul(
                                ps[ts * 2 + vg],
                                xt[c][:, ts * 128:(ts + 1) * 128],
                                wts[c][:, vg * 512:(vg + 1) * 512],
                                start=(c == 0), stop=(c == 15))
                for ts in range(4):
                    for vg in range(2):
                        ot = op.tile([128, 512], F32R, tag="o",
                                     name=f"vo{tb}_{ts}_{vg}")
                        if bvb_sb is not None:
                            nc.vector.tensor_add(
                                ot, ps[ts * 2 + vg],
                                bvb_sb[:, vg * 512:(vg + 1) * 512])
                        else:
                            nc.vector.tensor_copy(ot, ps[ts * 2 + vg])
                        nc.sync.dma_start(
                            out=v_s[tb * 512 + ts * 128:tb * 512 + (ts + 1) * 128,
                                    vg * 512:(vg + 1) * 512],
                            in_=ot)

        with tc.spectator_scope("p_k"):
            kq_proj(xeT, wk, kt_s, bk_b, "k")
        with tc.spectator_scope("p_v"):
            v_proj()

        with tc.spectator_scope("p_q"):
            kq_proj(xdT, wq, qt_s, bq_b, "q")
        proj.close()

        aexp = att.enter_context(tc.tile_pool(name="aex", bufs=10))
        adpp = att.enter_context(tc.tile_pool(name="adp", bufs=8))
        asmp = att.enter_context(tc.tile_pool(name="asm", bufs=3))
        aotp = att.enter_context(tc.tile_pool(name="aot", bufs=3))
        apsp = att.enter_context(tc.tile_pool(name="aps", bufs=1, space="PSUM"))

        # wo resident load (fills in during attention)
        wo_pool = ExitStack()
        wop = wo_pool.enter_context(tc.tile_pool(name="owo", bufs=1, side="right"))
        wo_sb = []
        for o in range(HPC):
            wt = wop.tile([128, DIM], F32R, tag=f"wo{o}", name=f"owo{o}")
            nc.sync.dma_start(out=wt, in_=wo[o * 128:(o + 1) * 128, :])
            wo_sb.append(wt)

        # ---- causal attention per head ----
        tri_mask = mask_sb[0][:, 0:128]
        with tc.spectator_scope("att"):
            for h in range(HPC):
                kt_h = []
                qt_h = []
                for j in range(n_tb):
                    ktj = aktp.tile([128, 512], F32R, tag="kt", name=f"akt{h}_{j}")
                    nc.sync.dma_start(
                        out=ktj,
                        in_=kt_s[h * 128:(h + 1) * 128, j * 512:(j + 1) * 512])
                    kt_h.append(ktj)
                    qtj = aqtp.tile([128, 512], F32R, tag="qt", name=f"aqt{h}_{j}")
                    nc.sync.dma_start(
                        out=qtj,
                        in_=qt_s[h * 128:(h + 1) * 128, j * 512:(j + 1) * 512])
                    qt_h.append(qtj)
                v_h = []
                for i in range(n_tc):
                    vt = avp.tile([128, 128], F32R, tag="v", name=f"av{h}_{i}")
                    nc.sync.dma_start(
                        out=vt,
                        in_=v_s[i * 128:(i + 1) * 128, h * 128:(h + 1) * 128])
                    v_h.append(vt)
                for g in range(n_g):
                    ni = 4 * g + 4  # k chunks 0..ni-1 are (partially) valid
                    n_full = 4 * g  # full-width (non-diagonal) chunks
                    # denominator ones-matmuls run on PE only for the 4
                    # diagonal chunks + one reduction of the DVE/GPS-built
                    # tree sum of the full-width exp tiles
                    n_dmm = 4 + (1 if n_full else 0)
                    dmm = 0
                    pso = apsp.tile([128, 512], F32, tag="po", name=f"apo{h}_{g}",
                                    bufs=2)
                    psd = apsp.tile([1, 512], F32, tag="pd", name=f"apd{h}_{g}",
                                    bufs=2)
                    full_ex = []
                    leaf_buf = []
                    for i in range(ni):
                        # diagonal chunks: columns < o are fully masked ->
                        # compute only the [o, 512) strip; the first 128 of
                        # the strip is the triangle that still needs masking
                        o = 128 * (i - 4 * g) if i >= 4 * g else 0
                        w = 512 - o
                        pss = apsp.tile([128, w], F32, tag="ps",
                                        name=f"aps{h}_{g}_{i}", bufs=3)
                        nc.tensor.matmul(
                            pss, kt_h[i // 4][:, (i % 4) * 128:(i % 4 + 1) * 128],
                            qt_h[g][:, o:512], start=True, stop=True)
                        ex = aexp.tile([128, w], F32R, tag="e",
                                       name=f"ae{h}_{g}_{i}")
                        nc.scalar.activation(ex, pss, AF.Exp, scale=float(SCALE))
                        if i >= 4 * g:
                            nc.vector.tensor_mul(ex[:, 0:128], ex[:, 0:128],
                                                 tri_mask)
                        nc.tensor.matmul(pso[:, o:512], v_h[i], ex,
                                         start=(i == 0), stop=(i == ni - 1))
                        if i >= 4 * g:
                            nc.tensor.matmul(psd[:, o:512], ones_sb, ex,
                                             start=(dmm == 0),
                                             stop=(dmm == n_dmm - 1))
                            dmm += 1
                        else:
                            leaf_buf.append(ex)
                            if len(leaf_buf) == 2:
                                # fold leaf pairs promptly so exp slots free
                                npair = len(full_ex)
                                dst = adpp.tile([128, 512], F32, tag="dp",
                                                name=f"adpL{h}_{g}_{npair}")
                                eng = nc.gpsimd if (npair % 2 == 0) else nc.vector
                                eng.tensor_add(dst, leaf_buf[0], leaf_buf[1])
                                full_ex.append(dst)
                                leaf_buf = []
                    # finish the tree on DVE; final add rounds to f32r
                    while len(full_ex) > 1:
                        last = len(full_ex) == 2
                        dst = adpp.tile([128, 512], F32R if last else F32,
                                        tag="dp",
                                        name=f"adpT{h}_{g}_{len(full_ex)}")
                        nc.vector.tensor_add(dst, full_ex[0], full_ex[1])
                        full_ex[:2] = [dst]
                    if full_ex:
                        fin = full_ex[0]
                        if fin.dtype != F32R:
                            fin2 = adpp.tile([128, 512], F32R, tag="dp",
                                             name=f"adpF{h}_{g}")
                            nc.vector.tensor_copy(fin2, fin)
                            fin = fin2
                        nc.tensor.matmul(psd, ones_sb, fin,
                                         start=False, stop=True)
                        dmm += 1
                    po_sb = aotp.tile([128, 512], F32, tag="posb",
                                      name=f"aposb{h}_{g}")
                    nc.vector.tensor_copy(po_sb, pso)
                    dinv_f = asmp.tile([1, 512], F32, tag="dif", name=f"adif{h}_{g}")
                    nc.vector.reciprocal_approx_fast(dinv_f, psd)
                    dinv = asmp.tile([1, 512], F32R, tag="di", name=f"adi{h}_{g}")
                    nc.vector.tensor_copy(dinv, dinv_f)
                    psb = apsp.tile([128, 512], F32, tag="db", name=f"adb{h}_{g}",
                                    bufs=1)
                    nc.tensor.matmul(psb, ones_r_sb, dinv, start=True, stop=True)
                    ot = aotp.tile([128, 512], F32R, tag="ot", name=f"aot{h}_{g}")
                    nc.vector.tensor_mul(ot, po_sb, psb)
                    nc.sync.dma_start(
                        out=ot_s[h * 128:(h + 1) * 128, g * 512:(g + 1) * 512],
                        in_=ot)
        att.close()

        # ---- output projection (partial: rows of W_o) ----
        with (
            tc.spectator_scope("oproj"),
            tc.tile_pool(name="ol", bufs=16) as olp,
            tc.tile_pool(name="oo", bufs=2) as oop,
            tc.tile_pool(name="ops", bufs=4, space="PSUM") as opp,
        ):
            for tch in range(n_tc):
                otl = []
                for o in range(HPC):
                    lt = olp.tile([128, 128], F32R, tag="l", name=f"ol{tch}_{o}")
                    nc.sync.dma_start(
                        out=lt,
                        in_=ot_s[o * 128:(o + 1) * 128, tch * 128:(tch + 1) * 128])
                    otl.append(lt)
                ps = [opp.tile([128, 512], F32, tag="p", name=f"op{tch}_{cg}")
                      for cg in range(4)]
                for o in range(HPC):
                    for cg in range(4):
                        nc.tensor.matmul(
                            ps[cg], otl[o], wo_sb[o][:, cg * 512:(cg + 1) * 512],
                            start=(o == 0), stop=(o == HPC - 1))
                osb = oop.tile([128, DIM], F32, tag="os", name=f"oo{tch}")
                for cg in range(4):
                    nc.vector.tensor_copy(osb[:, cg * 512:(cg + 1) * 512], ps[cg])
                nc.sync.dma_start(out=out[tch * 128:(tch + 1) * 128, :], in_=osb)
        wo_pool.close()

    nc.compile()
    return nc


def _host_masks():
    c = np.arange(512)[None, :]
    r = np.arange(128)[:, None]
    return np.stack([(c >= r + 128 * o).astype(np.float32) for o in range(4)])


def _make_in_maps(inputs):
    encoder_x = np.asarray(inputs["encoder_x"], dtype=np.float32)
    decoder_x = np.asarray(inputs["decoder_x"], dtype=np.float32)
    W_kv = np.asarray(inputs["W_kv"], dtype=np.float32)
    b_kv = np.asarray(inputs["b_kv"], dtype=np.float32)
    W_q = np.asarray(inputs["W_q"], dtype=np.float32)
    b_q = np.asarray(inputs["b_q"], dtype=np.float32)
    W_o = np.asarray(inputs["W_o"], dtype=np.float32)

    add_bias_kq = bool(np.any(b_kv) or np.any(b_q))
    masks = _host_masks()
    ones = np.ones((128, 1), np.float32)
    ones_r = np.ones((1, 128), np.float32)

    in_maps = []
    for core in range(N_CORES):
        b, hg = core // 2, core % 2
        s = hg * KC
        im = {
            "xeT": np.ascontiguousarray(encoder_x[b].T),
            "xdT": np.ascontiguousarray(decoder_x[b].T),
            "wk": np.ascontiguousarray(W_kv[:, s:s + KC]),
            "wv": np.ascontiguousarray(W_kv[:, DIM + s:DIM + s + KC]),
            "wq": np.ascontiguousarray(W_q[:, s:s + KC]),
            "wo": np.ascontiguousarray(W_o[s:s + KC, :]),
            "masks": masks,
            "ones": ones,
            "ones_r": ones_r,
        }
        if add_bias_kq:
            im["bk"] = np.ascontiguousarray(b_kv[s:s + KC][:, None])
            im["bq"] = np.ascontiguousarray(b_q[s:s + KC][:, None])
            im["bvb"] = np.ascontiguousarray(
                np.broadcast_to(b_kv[DIM + s:DIM + s + KC], (128, KC)))
        in_maps.append(im)
    return in_maps


def kernel(encoder_x, decoder_x, W_kv, b_kv, W_q, b_q, W_o, b_o):
    from concourse.bass_utils import run_bass_kernel_spmd

    b_kv = np.asarray(b_kv, dtype=np.float32)
    b_q = np.asarray(b_q, dtype=np.float32)
    b_o = np.asarray(b_o, dtype=np.float32)

    add_bias_kq = bool(np.any(b_kv) or np.any(b_q))
    nc = _build(T, add_bias_kq=add_bias_kq)

    in_maps = _make_in_maps(dict(
        encoder_x=encoder_x, decoder_x=decoder_x, W_kv=W_kv, b_kv=b_kv,
        W_q=W_q, b_q=b_q, W_o=W_o, b_o=b_o))

    res = run_bass_kernel_spmd(nc, in_maps, core_ids=list(range(N_CORES)),
                               trace=False)
    out = np.empty((B, T, DIM), np.float32)
    for b in range(B):
        out[b] = (res.results[2 * b]["out"].astype(np.float64)
                  + res.results[2 * b + 1]["out"].astype(np.float64)
                  + b_o.astype(np.float64)).astype(np.float32)
    return out

